# revision 21
# baseline (speedup 1.0000x reference)
"""BitNet transformer block kernel for 8 Trainium2 NeuronCores.

Sharding: data-parallel. Core c handles batch c//4, token chunk c%4 (512
query tokens). Each core computes K/V for its batch's full 2048-token
sequence (redundant KV compute instead of collectives). Host rotates the
token axis per core so every core's own tokens sit at chunk 0 -> all 8
cores run one identical SPMD program.

Layout: activations feature-major (x^T tiles [128 feat, T tok]) so matmul
contraction (features) lies on partitions. V is token-major with a ones
column appended per head so the AV matmul also produces the softmax
denominator. Whole datapath fp16 (fp32 PSUM accumulation); LN stats f32r.
Attention: softmax max/exp read logits directly from PSUM (no staging
copy), normalization folded into the av->ATTN write.
"""
import sys

sys.path.insert(0, "/opt/trn_rl_repo")

import numpy as np
from contextlib import ExitStack

import concourse.bass as bass
import concourse.bacc as bacc
import concourse.tile as tile
from concourse import mybir
from concourse.bass_utils import run_bass_kernel_spmd
from concourse.masks import make_identity

F32 = mybir.dt.float32
F32R = mybir.dt.float32r
F16 = mybir.dt.float16
AF = mybir.ActivationFunctionType
AX = mybir.AxisListType

DIM = 1024
HEADS = 16
DH = 64
FF = 4096
EPS = 1e-5
T = 2048        # tokens per batch (per-core KV scope)
NQ = 512        # own query tokens per core
KD = DIM // 128   # 8 feature tiles
CHUNK = 512
NCHUNK = T // CHUNK  # 4
N_CORES = 8

_cache = {}


def _quantize(w):
    w = w.astype(np.float32)
    return np.round(np.clip(w, -2.0, 2.0) * np.float32(0.75) + np.float32(0.5)) - np.float32(0.5)


def _prep_weights(i):
    """Host-side: quantize, fold scales/LN-params, transpose to [in, out]."""
    q = {k: _quantize(i[k]) for k in ("wq", "wk", "wv", "wo", "w1", "w2")}
    Wq = q["wq"] * i["sq"][:, None]
    Wk = q["wk"] * i["sk"][:, None]
    Wv = q["wv"] * i["sv"][:, None]
    Wo = q["wo"] * i["so"][:, None]
    W1 = q["w1"] * i["s1"][:, None]
    W2 = q["w2"] * i["s2"][:, None]
    g1, b1n = i["ln1_g"], i["ln1_b"]
    g2, b2n = i["ln2_g"], i["ln2_b"]
    s8 = np.float32(DH ** -0.5)
    out = {}
    out["wqT"] = np.ascontiguousarray((Wq * g1[None, :] * s8).T.astype(np.float16))
    out["bq"] = ((Wq @ b1n + i["bq"]) * s8).astype(np.float32)
    out["wkT"] = np.ascontiguousarray((Wk * g1[None, :]).T.astype(np.float16))
    out["bk"] = (Wk @ b1n + i["bk"]).astype(np.float32)
    out["wvT"] = np.ascontiguousarray((Wv * g1[None, :]).T.astype(np.float16))
    bv = Wv @ b1n + i["bv"]
    out["woT"] = np.ascontiguousarray(Wo.T.astype(np.float16))
    out["bo"] = (Wo @ bv + i["bo"]).astype(np.float32)
    out["w1T"] = np.ascontiguousarray((W1 * g2[None, :]).T.astype(np.float16))
    out["b1"] = (W1 @ b2n + i["b1"]).astype(np.float32)
    out["w2T"] = np.ascontiguousarray(W2.T.astype(np.float16))
    out["b2"] = i["b2"].astype(np.float32)
    return out


def _bcast_ap(t, n):
    """Partition-broadcast read AP of a [1, ...] sbuf/psum tile to n rows."""
    return bass.AP(tensor=t.tensor, offset=t.offset,
                   ap=[[0, n]] + [list(a) for a in t.ap[1:]])


def _ln_chunk(nc, sb, ps_bc, scratch, ps_stat, xh_pool, xt, ones_in, ones_sq,
              ones1, width, out_dt=F16):
    """LayerNorm transform of one feature-major chunk [128, KD, width].
    Returns xh = (x - mu) * rstd in out_dt. ones_in must match xt dtype."""
    ssum = ps_stat.tile([1, width], F32, name="ssum")
    ssq = ps_stat.tile([1, width], F32, name="ssq")
    for k in range(KD):
        sq = scratch.tile([128, width], F32R, name="scr", tag="scr")
        nc.scalar.activation(sq[:], xt[:, k], AF.Square)
        nc.tensor.matmul(ssum[:], lhsT=ones_in[:], rhs=xt[:, k],
                         start=(k == 0), stop=(k == KD - 1))
        nc.tensor.matmul(ssq[:], lhsT=ones_sq[:], rhs=sq[:],
                         start=(k == 0), stop=(k == KD - 1))
    mu = sb.tile([1, width], F32R, name="mu")
    nc.vector.tensor_scalar_mul(mu[:], ssum[:], 1.0 / DIM)
    var = sb.tile([1, width], F32, name="var")
    musq = sb.tile([1, width], F32, name="musq")
    nc.vector.tensor_mul(musq[:], mu[:], mu[:])
    nc.vector.tensor_scalar(var[:], ssq[:], 1.0 / DIM, None,
                            mybir.AluOpType.mult)
    nc.vector.tensor_sub(var[:], var[:], musq[:])
    nc.vector.tensor_scalar_add(var[:], var[:], float(EPS))
    sd = sb.tile([1, width], F32, name="sd")
    nc.scalar.activation(sd[:], var[:], AF.Sqrt)
    r = sb.tile([1, width], F32R, name="r")
    with nc.allow_low_precision(reason="f32r is fp32 storage"):
        nc.vector.reciprocal(r[:], sd[:])
    mu_b = ps_bc.tile([128, width], F32, name="mu_b")
    r_b = ps_bc.tile([128, width], F32, name="r_b")
    nc.tensor.matmul(mu_b[:], lhsT=ones1[:], rhs=mu[:], start=True, stop=True)
    nc.tensor.matmul(r_b[:], lhsT=ones1[:], rhs=r[:], start=True, stop=True)
    xh = xh_pool.tile([128, KD, width], out_dt, name="xh")
    for k in range(KD):
        xc = scratch.tile([128, width], F32, name="scr2", tag="scr")
        nc.vector.tensor_sub(xc[:], xt[:, k], mu_b[:])
        nc.vector.tensor_mul(xh[:, k], xc[:], r_b[:])
    return xh


def _wslice(d, name, m, mm=128):
    """[DIM_in, n_out] weight dram -> lhsT tile view [128, KD_in, mm] for
    out-block m."""
    return d[name].rearrange("(k p) (mb mm) -> p k mb mm", p=128, mm=mm)[:, :, m]


def _phase_a(nc, tc, d, const, KT_sb, V_sb, QT_sb, xt0, bias, ones16, ones32, ones1):
    """LN1 + K/V proj over all 4 chunks; Q proj on chunk 0."""
    xT_t = d["xT"].rearrange("(k p) t -> p k t", p=128)
    with ExitStack() as actx:
        sb_ln = actx.enter_context(tc.tile_pool(name="sb_ln", bufs=2))
        ps_bc = actx.enter_context(tc.tile_pool(name="ps_bc", bufs=1, space="PSUM"))
        scratch = actx.enter_context(tc.tile_pool(name="scratch", bufs=2))
        sb_xt = actx.enter_context(tc.tile_pool(name="sb_xt", bufs=2))
        sb_xh = actx.enter_context(tc.tile_pool(name="sb_xh", bufs=2))
        wstr = actx.enter_context(tc.tile_pool(name="wstr", bufs=2))
        ps_stat = actx.enter_context(tc.tile_pool(name="ps_stat", bufs=1, space="PSUM"))
        ps_mm = actx.enter_context(tc.tile_pool(name="ps_mm", bufs=4, space="PSUM"))

        for c in range(NCHUNK):
            if c == 0:
                xt = xt0
            else:
                xt = sb_xt.tile([128, KD, CHUNK], F16, name="xt")
            nc.sync.dma_start(out=xt[:], in_=xT_t[:, :, c * CHUNK:(c + 1) * CHUNK])
            xh = _ln_chunk(nc, sb_ln, ps_bc, scratch, ps_stat, sb_xh, xt,
                           ones16, ones32, ones1, CHUNK)

            # K projection (feature-major out)
            for m in range(KD):
                wk = wstr.tile([128, KD, 128], F16, name="wk", tag="wk")
                nc.sync.dma_start(out=wk[:], in_=_wslice(d, "wkT", m))
                kp = ps_mm.tile([128, CHUNK], F32, name="kp", tag="mm")
                for k in range(KD):
                    nc.tensor.matmul(kp[:], lhsT=wk[:, k], rhs=xh[:, k],
                                     start=(k == 0), stop=(k == KD - 1))
                nc.scalar.activation(KT_sb[:, m, c * CHUNK:(c + 1) * CHUNK], kp[:],
                                     AF.Identity, bias=bias["bk"][:, m:m + 1])
            # V projection (token-major out, ones col per head at offset 64)
            wvT_v = d["wvT"].rearrange("(kh k p) (nb nn) -> p kh k nb nn",
                                       p=128, k=4, nn=CHUNK)
            for nb in range(2):
                wvs = []
                for kh in range(2):
                    wv = wstr.tile([128, 4, CHUNK], F16, name="wv", tag="wv")
                    nc.sync.dma_start(out=wv[:], in_=wvT_v[:, kh, :, nb])
                    wvs.append(wv)
                for t_sub in range(CHUNK // 128):
                    blk = c * 4 + t_sub
                    vp = ps_mm.tile([128, CHUNK], F32, name="vp", tag="mm")
                    for k in range(KD):
                        nc.tensor.matmul(
                            vp[:], lhsT=xh[:, k, t_sub * 128:(t_sub + 1) * 128],
                            rhs=wvs[k // 4][:, k % 4], start=(k == 0), stop=(k == KD - 1))
                    nc.scalar.copy(
                        V_sb[:, blk, nb * CHUNK:(nb + 1) * CHUNK], vp[:])
            if c == 0:
                for m in range(KD):
                    wq = wstr.tile([128, KD, 128], F16, name="wq", tag="wk")
                    nc.sync.dma_start(out=wq[:], in_=_wslice(d, "wqT", m))
                    qp = ps_mm.tile([128, CHUNK], F32, name="qp", tag="mm")
                    for k in range(KD):
                        nc.tensor.matmul(qp[:], lhsT=wq[:, k], rhs=xh[:, k],
                                         start=(k == 0), stop=(k == KD - 1))
                    nc.scalar.activation(QT_sb[:, m], qp[:],
                                         AF.Identity, bias=bias["bq"][:, m:m + 1])


def _phase_b(nc, tc, KT_sb, V_sb, QT_sb, ATTN_mbs, ident, ident_f32):
    """Attention: q-major logits, PSUM-direct softmax (max+exp read PSUM),
    per-head normalization applied to the small AV output."""
    units = [(h, qt) for h in range(HEADS) for qt in range(NQ // 128)]
    with ExitStack() as bctx:
        ps_S = bctx.enter_context(tc.tile_pool(name="ps_S", bufs=3, space="PSUM"))
        ps_tp = bctx.enter_context(tc.tile_pool(name="ps_tp", bufs=1, space="PSUM"))
        ps_av = bctx.enter_context(tc.tile_pool(name="ps_av", bufs=1, space="PSUM"))
        sb_A = bctx.enter_context(tc.tile_pool(name="sb_A", bufs=4))
        sb_AT = bctx.enter_context(tc.tile_pool(name="sb_AT", bufs=2))
        sb_st = bctx.enter_context(tc.tile_pool(name="sb_st", bufs=4))
        sb_rd = bctx.enter_context(tc.tile_pool(name="sb_rd", bufs=2))

        state = {}   # i -> (A tile, AT tile, den4 tile, h, qt)
        AT_cur = None
        den4_cur = None

        def front_half(i, half):
            """QK + partial maxes for one 1024-key half of unit i."""
            nonlocal AT_cur, den4_cur
            h, qt = units[i]
            mb, r0 = h // 2, (h % 2) * 64
            if half == 0:
                if qt == 0:
                    AT_cur = sb_AT.tile([128, T // 128, NQ], F16, name="AT")
                    den4_cur = sb_st.tile([128, 4], F32, name="den4", tag="den4")
                A = sb_A.tile([128, T], F16, name="A")
                mx = sb_st.tile([128, 2], F32, name="mx", tag="mx")
                state[i] = [A, AT_cur, den4_cur, h, qt, mx, [None, None]]
            A, AT, den4, h, qt, mx, halves = state[i]
            q_sl = QT_sb[r0:r0 + 64, mb, qt * 128:(qt + 1) * 128]
            S = ps_S.tile([128, 2, CHUNK], F32, name="S")
            halves[half] = S
            for j in range(2):
                k_off = (half * 2 + j) * CHUNK
                nc.tensor.matmul(
                    S[:, j], lhsT=q_sl,
                    rhs=KT_sb[r0:r0 + 64, mb, k_off:k_off + CHUNK],
                    start=True, stop=True)
            nc.vector.reduce_max(mx[:, half:half + 1], S[:], axis=AX.XY)
            if half == 1:
                nM = sb_st.tile([128, 1], F32, name="nM", tag="nM")
                nc.vector.reduce_max(nM[:], mx[:], axis=AX.X, negate=True)
                hd0 = sb_st.tile([128, 1], F32, name="hd0", tag="hd0")
                hd1 = sb_st.tile([128, 1], F32, name="hd1", tag="hd1")
                nc.scalar.activation(
                    A[:, 0:T // 2], halves[0].rearrange("p a b -> p (a b)"),
                    AF.Exp, bias=nM[:], accum_out=hd0[:])
                nc.scalar.activation(
                    A[:, T // 2:T], halves[1].rearrange("p a b -> p (a b)"),
                    AF.Exp, bias=nM[:], accum_out=hd1[:])
                nc.gpsimd.tensor_add(den4[:, qt:qt + 1], hd0[:], hd1[:])

        tail2_state = {}

        def tail1_half(i, half):
            """Transpose A -> AT via XBAR DMA; AV at head end."""
            A, AT, den4, h, qt, mx, halves = state[i]
            mb, r0 = h // 2, (h % 2) * 64
            dst = AT[:, half * 8:(half + 1) * 8, qt * 128:(qt + 1) * 128]
            nc.sync.dma_start(
                out=dst, in_=A[:, half * (T // 2):(half + 1) * (T // 2)],
                transpose=True)
            if half == 0:
                return
            state.pop(i)
            if qt == NQ // 128 - 1:
                # rden: [128q,4qt] -> transpose -> [1, 512] -> bcast [128, 512]
                rd4 = sb_st.tile([128, 4], F32, name="rd4", tag="rd4")
                nc.vector.reciprocal(rd4[:], den4[:])
                rdT_ps = ps_tp.tile([1, 4, 128], F32, name="rdT", tag="tp")
                for q4 in range(4):
                    nc.tensor.transpose(rdT_ps[:, q4], rd4[:, q4:q4 + 1],
                                        ident_f32[:])
                rdT = sb_rd.tile([1, NQ], F32, name="rdT_sb", tag="rdT")
                nc.vector.tensor_copy(rdT[:], rdT_ps.rearrange("p a b -> p (a b)"))
                rd_b = sb_rd.tile([128, NQ], F32, name="rd_b", tag="rd_b")
                nc.gpsimd.partition_broadcast(rd_b[:], rdT[:])
                av = ps_av.tile([128, NQ], F32, name="av")
                for kb in range(T // 128):
                    nc.tensor.matmul(av[r0:r0 + DH, :],
                                     lhsT=V_sb[:, kb, h * DH:(h + 1) * DH],
                                     rhs=AT[:, kb],
                                     start=(kb == 0), stop=(kb == T // 128 - 1))
                tail2_state[i] = (av, rd_b, mb, r0)

        def emit_tail2(i):
            if i not in tail2_state:
                return
            av, rd_b, mb, r0 = tail2_state.pop(i)
            nc.vector.tensor_mul(ATTN_mbs[mb][r0:r0 + 64, :],
                                 av[r0:r0 + DH, :], rd_b[r0:r0 + DH, :])

        n = len(units)
        for i in range(n + 2):
            if i < n:
                front_half(i, 0)
            if 0 <= i - 1 < n:
                tail1_half(i - 1, 0)
            if i < n:
                front_half(i, 1)
            if 0 <= i - 1 < n:
                tail1_half(i - 1, 1)
            if i > 1:
                emit_tail2(i - 2)


def _phase_c(nc, tc, d, ATTN_mbs, xt0, bias, ones32, ones1, wo_sb, w1_sb):
    """O proj + residual + LN2 + FF + output store."""
    with ExitStack() as cctx:
        sb_ln2 = cctx.enter_context(tc.tile_pool(name="sb_ln2", bufs=2))
        ps_bc2 = cctx.enter_context(tc.tile_pool(name="ps_bc2", bufs=1, space="PSUM"))
        scr2 = cctx.enter_context(tc.tile_pool(name="scr2", bufs=2))
        sb_u = cctx.enter_context(tc.tile_pool(name="sb_u", bufs=1))
        wstr2 = cctx.enter_context(tc.tile_pool(name="wstr2", bufs=4))
        ps_stat2 = cctx.enter_context(tc.tile_pool(name="ps_stat2", bufs=1, space="PSUM"))
        ps_mm2 = cctx.enter_context(tc.tile_pool(name="ps_mm2", bufs=4, space="PSUM"))

        u_sb = sb_u.tile([128, KD, NQ], F32R, name="u_sb")
        wo_v = wo_sb.rearrange("p (k mb mm) -> p k mb mm", k=KD, mm=128)
        for m in range(KD):
            op = ps_mm2.tile([128, NQ], F32, name="op", tag="mm")
            for k in range(KD):
                nc.tensor.matmul(op[:], lhsT=wo_v[:, k, m], rhs=ATTN_mbs[k][:],
                                 start=(k == 0), stop=(k == KD - 1))
            upre = scr2.tile([128, NQ], F32, name="upre", tag="scr")
            nc.vector.tensor_add(upre[:], op[:], xt0[:, m])
            nc.scalar.activation(u_sb[:, m], upre[:], AF.Identity,
                                 bias=bias["bo"][:, m:m + 1])
        uh = _ln_chunk(nc, sb_ln2, ps_bc2, scr2, ps_stat2, sb_u, u_sb, ones32,
                       ones32, ones1, NQ, out_dt=F16)
        H_sb = sb_u.tile([128, FF // 128, NQ], F16, name="H_sb")
        w1_v = w1_sb.rearrange("p (k mb mm) -> p k mb mm", k=KD, mm=128)
        NPRE = (FF // 128) // 2  # m-blocks prefetched in w1_sb
        for m in range(FF // 128):
            if m < NPRE:
                w1 = w1_v[:, :, m]
            else:
                w1t = wstr2.tile([128, KD, 128], F16, name="w1", tag="wsm")
                nc.sync.dma_start(out=w1t[:], in_=_wslice(d, "w1T", m))
                w1 = w1t
            h1 = ps_mm2.tile([128, NQ], F32, name="h1", tag="mm")
            for k in range(KD):
                nc.tensor.matmul(h1[:], lhsT=w1[:, k], rhs=uh[:, k],
                                 start=(k == 0), stop=(k == KD - 1))
            nc.scalar.activation(H_sb[:, m], h1[:], AF.Gelu,
                                 bias=bias["b1"][:, m:m + 1])
        w2T_v = d["w2T"].rearrange("(kh k p) (mb mm) -> p kh k mb mm",
                                   p=128, k=8, mm=128)
        for m in range(KD):
            f2 = ps_mm2.tile([128, NQ], F32, name="f2", tag="mm")
            for kh in range(4):
                w2 = wstr2.tile([128, 8, 128], F16, name="w2", tag="w2")
                nc.sync.dma_start(out=w2[:], in_=w2T_v[:, kh, :, m])
                for k in range(8):
                    nc.tensor.matmul(f2[:], lhsT=w2[:, k], rhs=H_sb[:, kh * 8 + k],
                                     start=(kh == 0 and k == 0),
                                     stop=(kh == 3 and k == 7))
            opre = scr2.tile([128, NQ], F32, name="opre", tag="scr")
            nc.vector.tensor_add(opre[:], f2[:], u_sb[:, m])
            oout = scr2.tile([128, NQ], F32, name="oout", tag="scr")
            nc.scalar.activation(oout[:], opre[:], AF.Identity,
                                 bias=bias["b2"][:, m:m + 1])
            nc.sync.dma_start(out=d["yT"][m * 128:(m + 1) * 128, :], in_=oout[:])


def _body(nc, tc, d):
    ctx = ExitStack()
    with ctx:
        const = ctx.enter_context(tc.tile_pool(name="const", bufs=1))
        ones_blk = const.tile([128, 128], F32, name="ones_blk")
        nc.vector.memset(ones_blk[:], 1.0)
        ones1r = const.tile([1, 128], F32R, name="ones1r")
        nc.vector.tensor_copy(ones1r[:], ones_blk[0:1, :])
        ones1 = ones1r[:]
        ones32 = const.tile([128, 1], F32R, name="ones32")
        nc.vector.tensor_copy(ones32[:], ones_blk[:, 0:1])
        ones16 = const.tile([128, 1], F16, name="ones16")
        nc.vector.tensor_copy(ones16[:], ones_blk[:, 0:1])
        ident = const.tile([128, 128], F16, name="ident")
        make_identity(nc, ident)
        ident_f32 = const.tile([128, 128], F32, name="ident_f32")
        make_identity(nc, ident_f32)

        bias = {}
        for nm, n in [("bq", DIM), ("bk", DIM), ("bo", DIM), ("b1", FF), ("b2", DIM)]:
            t = const.tile([128, n // 128], F32, name=f"sb_{nm}")
            nc.sync.dma_start(out=t[:], in_=d[nm].rearrange("(m p) -> p m", p=128))
            bias[nm] = t

        # long-lived activations
        xt0 = const.tile([128, KD, CHUNK], F16, name="xt0")
        ATTN_mbs = [const.tile([128, NQ], F16, name=f"ATTN_{i}") for i in range(KD)]

        # phase-C weights, prefetched during attention (issued after phase A
        # so they do not delay phase A's own DMA traffic)
        wpre = ctx.enter_context(tc.tile_pool(name="wpre", bufs=1))
        wo_sb = wpre.tile([128, KD * DIM], F16, name="wo_sb")
        w1_sb = wpre.tile([128, KD * (FF // 2)], F16, name="w1_sb")

        with tc.tile_pool(name="attn_mem", bufs=1) as am:
            KT_sb = am.tile([128, KD, T], F16, name="KT_sb")              # 4MB
            V_sb = am.tile([128, T // 128, DIM], F16, name="V_sb")
            QT_sb = am.tile([128, KD, NQ], F16, name="QT_sb")

            _phase_a(nc, tc, d, const, KT_sb, V_sb, QT_sb, xt0, bias, ones16, ones32, ones1)
            nc.sync.dma_start(out=wo_sb.rearrange("p (k n) -> p k n", k=KD),
                              in_=d["woT"].rearrange("(k p) n -> p k n", p=128))
            nc.sync.dma_start(
                out=w1_sb.rearrange("p (k n) -> p k n", k=KD),
                in_=d["w1T"].rearrange("(k p) n -> p k n", p=128)[:, :, 0:FF // 2])
            _phase_b(nc, tc, KT_sb, V_sb, QT_sb, ATTN_mbs, ident, ident_f32)
        _phase_c(nc, tc, d, ATTN_mbs, xt0, bias, ones32, ones1, wo_sb, w1_sb)


def _build():
    nc = bacc.Bacc("TRN2", target_bir_lowering=False, debug=False,
                   num_devices=N_CORES)
    d = {}
    d["xT"] = nc.dram_tensor("xT", [DIM, T], F16, kind="ExternalInput").ap()
    d["wqT"] = nc.dram_tensor("wqT", [DIM, DIM], F16, kind="ExternalInput").ap()
    d["wkT"] = nc.dram_tensor("wkT", [DIM, DIM], F16, kind="ExternalInput").ap()
    d["wvT"] = nc.dram_tensor("wvT", [DIM, DIM], F16, kind="ExternalInput").ap()
    d["woT"] = nc.dram_tensor("woT", [DIM, DIM], F16, kind="ExternalInput").ap()
    d["w1T"] = nc.dram_tensor("w1T", [DIM, FF], F16, kind="ExternalInput").ap()
    d["w2T"] = nc.dram_tensor("w2T", [FF, DIM], F16, kind="ExternalInput").ap()
    for nm, n in [("bq", DIM), ("bk", DIM), ("bo", DIM), ("b1", FF), ("b2", DIM)]:
        d[nm] = nc.dram_tensor(nm, [n], F32, kind="ExternalInput").ap()
    d["yT"] = nc.dram_tensor("yT", [DIM, NQ], F32, kind="ExternalOutput").ap()
    with tile.TileContext(nc) as tc:
        _body(nc, tc, d)
    nc.compile()
    return nc


def _in_maps(inputs):
    x = inputs["x"].astype(np.float32)
    B = x.shape[0]
    w = _prep_weights(inputs)
    per_batch = N_CORES // B
    maps = []
    for c in range(N_CORES):
        b, chunk = divmod(c, per_batch)
        xT = np.ascontiguousarray(np.roll(x[b].T, -chunk * NQ, axis=1)).astype(np.float16)
        m = {"xT": xT}
        m.update(w)
        maps.append(m)
    return maps


def kernel(**inputs) -> np.ndarray:
    inputs = {k: np.asarray(v) for k, v in inputs.items()}
    x = inputs["x"].astype(np.float32)
    B, N, D = x.shape  # (2, 2048, 1024)

    if "nc" not in _cache:
        _cache["nc"] = _build()
    nc = _cache["nc"]

    res = run_bass_kernel_spmd(nc, _in_maps(inputs), core_ids=list(range(N_CORES)))
    per_batch = N_CORES // B
    out = np.empty((B, N, D), dtype=np.float32)
    for c in range(N_CORES):
        b, chunk = divmod(c, per_batch)
        out[b, chunk * NQ:(chunk + 1) * NQ, :] = res.results[c]["yT"].T
    return out


# revision 24
# speedup vs baseline: 1.1604x; 1.1604x over previous
"""BitNet transformer block kernel for 8 Trainium2 NeuronCores.

Sharding: data-parallel. Core c handles batch c//4, token chunk c%4 (512
query tokens). Each core computes K/V for its batch's full 2048-token
sequence (redundant KV compute instead of collectives). Host rotates the
token axis per core so every core's own tokens sit at chunk 0 -> all 8
cores run one identical SPMD program.

Layout: activations feature-major (x^T tiles [128 feat, T tok]) so matmul
contraction (features) lies on partitions. V is token-major with a ones
column appended per head so the AV matmul also produces the softmax
denominator. Whole datapath fp16 (fp32 PSUM accumulation); LN stats f32r.
Attention: softmax max/exp read logits directly from PSUM (no staging
copy), normalization folded into the av->ATTN write.
"""
import sys

sys.path.insert(0, "/opt/trn_rl_repo")

import numpy as np
from contextlib import ExitStack

import concourse.bass as bass
import concourse.bacc as bacc
import concourse.tile as tile
from concourse import mybir
from concourse.bass_utils import run_bass_kernel_spmd
from concourse.masks import make_identity

F32 = mybir.dt.float32
F32R = mybir.dt.float32r
F16 = mybir.dt.float16
AF = mybir.ActivationFunctionType
AX = mybir.AxisListType

DIM = 1024
HEADS = 16
DH = 64
FF = 4096
EPS = 1e-5
T = 2048        # tokens per batch (per-core KV scope)
NQ = 512        # own query tokens per core
KD = DIM // 128   # 8 feature tiles
CHUNK = 512
NCHUNK = T // CHUNK  # 4
N_CORES = 8

_cache = {}


def _quantize(w):
    w = w.astype(np.float32)
    return np.round(np.clip(w, -2.0, 2.0) * np.float32(0.75) + np.float32(0.5)) - np.float32(0.5)


def _prep_weights(i):
    """Host-side: quantize, fold scales/LN-params, transpose to [in, out]."""
    q = {k: _quantize(i[k]) for k in ("wq", "wk", "wv", "wo", "w1", "w2")}
    Wq = q["wq"] * i["sq"][:, None]
    Wk = q["wk"] * i["sk"][:, None]
    Wv = q["wv"] * i["sv"][:, None]
    Wo = q["wo"] * i["so"][:, None]
    W1 = q["w1"] * i["s1"][:, None]
    W2 = q["w2"] * i["s2"][:, None]
    g1, b1n = i["ln1_g"], i["ln1_b"]
    g2, b2n = i["ln2_g"], i["ln2_b"]
    s8 = np.float32(DH ** -0.5)
    out = {}
    out["wqT"] = np.ascontiguousarray((Wq * g1[None, :] * s8).T.astype(np.float16))
    out["bq"] = ((Wq @ b1n + i["bq"]) * s8).astype(np.float32)
    out["wkT"] = np.ascontiguousarray((Wk * g1[None, :]).T.astype(np.float16))
    out["bk"] = (Wk @ b1n + i["bk"]).astype(np.float32)
    out["wvT"] = np.ascontiguousarray((Wv * g1[None, :]).T.astype(np.float16))
    bv = Wv @ b1n + i["bv"]
    out["woT"] = np.ascontiguousarray(Wo.T.astype(np.float16))
    out["bo"] = (Wo @ bv + i["bo"]).astype(np.float32)
    out["w1T"] = np.ascontiguousarray((W1 * g2[None, :]).T.astype(np.float16))
    out["b1"] = (W1 @ b2n + i["b1"]).astype(np.float32)
    out["w2T"] = np.ascontiguousarray(W2.T.astype(np.float16))
    out["b2"] = i["b2"].astype(np.float32)
    return out


def _bcast_ap(t, n):
    """Partition-broadcast read AP of a [1, ...] sbuf/psum tile to n rows."""
    return bass.AP(tensor=t.tensor, offset=t.offset,
                   ap=[[0, n]] + [list(a) for a in t.ap[1:]])


def _ln_chunk(nc, sb, ps_bc, scratch, ps_stat, xh_pool, xt, ones_in, ones_sq,
              ones1, width, out_dt=F16):
    """LayerNorm transform of one feature-major chunk [128, KD, width].
    Returns xh = (x - mu) * rstd in out_dt. ones_in must match xt dtype."""
    ssum = ps_stat.tile([1, width], F32, name="ssum")
    ssq = ps_stat.tile([1, width], F32, name="ssq")
    for k in range(KD):
        sq = scratch.tile([128, width], F32R, name="scr", tag="scr")
        nc.scalar.activation(sq[:], xt[:, k], AF.Square)
        nc.tensor.matmul(ssum[:], lhsT=ones_in[:], rhs=xt[:, k],
                         start=(k == 0), stop=(k == KD - 1))
        nc.tensor.matmul(ssq[:], lhsT=ones_sq[:], rhs=sq[:],
                         start=(k == 0), stop=(k == KD - 1))
    mu = sb.tile([1, width], F32R, name="mu")
    nc.vector.tensor_scalar_mul(mu[:], ssum[:], 1.0 / DIM)
    var = sb.tile([1, width], F32, name="var")
    musq = sb.tile([1, width], F32, name="musq")
    nc.vector.tensor_mul(musq[:], mu[:], mu[:])
    nc.vector.tensor_scalar(var[:], ssq[:], 1.0 / DIM, None,
                            mybir.AluOpType.mult)
    nc.vector.tensor_sub(var[:], var[:], musq[:])
    nc.vector.tensor_scalar_add(var[:], var[:], float(EPS))
    sd = sb.tile([1, width], F32, name="sd")
    nc.scalar.activation(sd[:], var[:], AF.Sqrt)
    r = sb.tile([1, width], F32R, name="r")
    with nc.allow_low_precision(reason="f32r is fp32 storage"):
        nc.vector.reciprocal(r[:], sd[:])
    mu_b = ps_bc.tile([128, width], F32, name="mu_b")
    r_b = ps_bc.tile([128, width], F32, name="r_b")
    nc.tensor.matmul(mu_b[:], lhsT=ones1[:], rhs=mu[:], start=True, stop=True)
    nc.tensor.matmul(r_b[:], lhsT=ones1[:], rhs=r[:], start=True, stop=True)
    xh = xh_pool.tile([128, KD, width], out_dt, name="xh")
    for k in range(KD):
        xc = scratch.tile([128, width], F32, name="scr2", tag="scr")
        nc.vector.tensor_sub(xc[:], xt[:, k], mu_b[:])
        nc.vector.tensor_mul(xh[:, k], xc[:], r_b[:])
    return xh


def _wslice(d, name, m, mm=128):
    """[DIM_in, n_out] weight dram -> lhsT tile view [128, KD_in, mm] for
    out-block m."""
    return d[name].rearrange("(k p) (mb mm) -> p k mb mm", p=128, mm=mm)[:, :, m]


def _phase_a(nc, tc, d, const, KT_sb, V_sb, QT_sb, xt0, bias, ones16, ones32, ones1):
    """LN1 + K/V proj over all 4 chunks; Q proj on chunk 0."""
    xT_t = d["xT"].rearrange("(k p) t -> p k t", p=128)
    with ExitStack() as actx:
        sb_ln = actx.enter_context(tc.tile_pool(name="sb_ln", bufs=2))
        ps_bc = actx.enter_context(tc.tile_pool(name="ps_bc", bufs=1, space="PSUM"))
        scratch = actx.enter_context(tc.tile_pool(name="scratch", bufs=2))
        sb_xt = actx.enter_context(tc.tile_pool(name="sb_xt", bufs=2))
        sb_xh = actx.enter_context(tc.tile_pool(name="sb_xh", bufs=2))
        wstr = actx.enter_context(tc.tile_pool(name="wstr", bufs=2))
        ps_stat = actx.enter_context(tc.tile_pool(name="ps_stat", bufs=1, space="PSUM"))
        ps_mm = actx.enter_context(tc.tile_pool(name="ps_mm", bufs=4, space="PSUM"))

        for c in range(NCHUNK):
            if c == 0:
                xt = xt0
            else:
                xt = sb_xt.tile([128, KD, CHUNK], F16, name="xt")
            nc.sync.dma_start(out=xt[:], in_=xT_t[:, :, c * CHUNK:(c + 1) * CHUNK])
            xh = _ln_chunk(nc, sb_ln, ps_bc, scratch, ps_stat, sb_xh, xt,
                           ones16, ones32, ones1, CHUNK)

            # K projection (feature-major out)
            for m in range(KD):
                wk = wstr.tile([128, KD, 128], F16, name="wk", tag="wk")
                nc.sync.dma_start(out=wk[:], in_=_wslice(d, "wkT", m))
                kp = ps_mm.tile([128, CHUNK], F32, name="kp", tag="mm")
                for k in range(KD):
                    nc.tensor.matmul(kp[:], lhsT=wk[:, k], rhs=xh[:, k],
                                     start=(k == 0), stop=(k == KD - 1))
                nc.scalar.activation(KT_sb[:, m, c * CHUNK:(c + 1) * CHUNK], kp[:],
                                     AF.Identity, bias=bias["bk"][:, m:m + 1])
            # V projection (token-major out, ones col per head at offset 64)
            wvT_v = d["wvT"].rearrange("(kh k p) (nb nn) -> p kh k nb nn",
                                       p=128, k=4, nn=CHUNK)
            for nb in range(2):
                wvs = []
                for kh in range(2):
                    wv = wstr.tile([128, 4, CHUNK], F16, name="wv", tag="wv")
                    nc.sync.dma_start(out=wv[:], in_=wvT_v[:, kh, :, nb])
                    wvs.append(wv)
                for t_sub in range(CHUNK // 128):
                    blk = c * 4 + t_sub
                    vp = ps_mm.tile([128, CHUNK], F32, name="vp", tag="mm")
                    for k in range(KD):
                        nc.tensor.matmul(
                            vp[:], lhsT=xh[:, k, t_sub * 128:(t_sub + 1) * 128],
                            rhs=wvs[k // 4][:, k % 4], start=(k == 0), stop=(k == KD - 1))
                    nc.scalar.copy(
                        V_sb[:, blk, nb * CHUNK:(nb + 1) * CHUNK], vp[:])
            if c == 0:
                for m in range(KD):
                    wq = wstr.tile([128, KD, 128], F16, name="wq", tag="wk")
                    nc.sync.dma_start(out=wq[:], in_=_wslice(d, "wqT", m))
                    qp = ps_mm.tile([128, CHUNK], F32, name="qp", tag="mm")
                    for k in range(KD):
                        nc.tensor.matmul(qp[:], lhsT=wq[:, k], rhs=xh[:, k],
                                         start=(k == 0), stop=(k == KD - 1))
                    nc.scalar.activation(QT_sb[:, m], qp[:],
                                         AF.Identity, bias=bias["bq"][:, m:m + 1])


def _phase_b(nc, tc, KT_sb, V_sb, QT_sb, ATTN_mbs, ident, ident_f32):
    """Attention: q-major logits, PSUM-direct softmax (max+exp read PSUM),
    per-head normalization applied to the small AV output."""
    units = [(h, qt) for h in range(HEADS) for qt in range(NQ // 128)]
    with ExitStack() as bctx:
        ps_s1 = bctx.enter_context(tc.tile_pool(name="ps_s1", bufs=3, space="PSUM"))
        ps_S = bctx.enter_context(tc.tile_pool(name="ps_S", bufs=2, space="PSUM"))
        ps_av = bctx.enter_context(tc.tile_pool(name="ps_av", bufs=1, space="PSUM"))
        sb_A = bctx.enter_context(tc.tile_pool(name="sb_A", bufs=4))
        sb_AT = bctx.enter_context(tc.tile_pool(name="sb_AT", bufs=2))
        sb_st = bctx.enter_context(tc.tile_pool(name="sb_st", bufs=4))
        sb_rd = bctx.enter_context(tc.tile_pool(name="sb_rd", bufs=2))

        state = {}   # i -> (A tile, AT tile, den4 tile, h, qt)
        AT_cur = None
        den4_cur = None

        def front_half(i, half):
            """QK + partial maxes for one 1024-key half of unit i."""
            nonlocal AT_cur, den4_cur
            h, qt = units[i]
            mb, r0 = h // 2, (h % 2) * 64
            if half == 0:
                if qt == 0:
                    AT_cur = sb_AT.tile([128, T // 128, NQ], F16, name="AT")
                    den4_cur = sb_st.tile([128, 4], F32, name="den4", tag="den4")
                A = sb_A.tile([128, T], F16, name="A")
                mx = sb_st.tile([128, 4], F32, name="mx", tag="mx")
                state[i] = [A, AT_cur, den4_cur, h, qt, mx, [None, None]]
            A, AT, den4, h, qt, mx = state[i][:6]
            q_sl = QT_sb[r0:r0 + 64, mb, qt * 128:(qt + 1) * 128]
            if half == 0:
                # pass 1: compute logits chunk-wise only to extract the max;
                # tiles are freed right after each reduce
                for c in range(4):
                    S1 = ps_s1.tile([128, CHUNK], F32, name="S1", tag="s1")
                    nc.tensor.matmul(
                        S1[:], lhsT=q_sl,
                        rhs=KT_sb[r0:r0 + 64, mb, c * CHUNK:(c + 1) * CHUNK],
                        start=True, stop=True)
                    nc.vector.reduce_max(mx[:, c:c + 1], S1[:], axis=AX.X)
                nM = sb_st.tile([128, 1], F32, name="nM", tag="nM")
                nc.vector.reduce_max(nM[:], mx[:], axis=AX.X, negate=True)
                state[i].append(nM)   # idx 7
            nM = state[i][7]
            # pass 2: recompute logits, exp with known bias
            S = ps_S.tile([128, 2, CHUNK], F32, name="S")
            for j in range(2):
                k_off = (half * 2 + j) * CHUNK
                nc.tensor.matmul(
                    S[:, j], lhsT=q_sl,
                    rhs=KT_sb[r0:r0 + 64, mb, k_off:k_off + CHUNK],
                    start=True, stop=True)
            hd = sb_st.tile([128, 1], F32, name=f"hd{half}", tag=f"hd{half}")
            nc.scalar.activation(
                A[:, half * (T // 2):(half + 1) * (T // 2)],
                S.rearrange("p a b -> p (a b)"),
                AF.Exp, bias=nM[:], accum_out=hd[:])
            state[i].append(hd)       # idx 8 (hd0), idx 9 (hd1)
            if half == 1:
                nc.gpsimd.tensor_add(den4[:, qt:qt + 1],
                                     state[i][8][:], state[i][9][:])

        tail2_state = {}

        def tail1_half(i, half):
            """Transpose A -> AT via XBAR DMA; AV at head end."""
            if half == 0:
                return
            A, AT, den4, h, qt = state[i][:5]
            mb, r0 = h // 2, (h % 2) * 64
            nc.sync.dma_start(
                out=AT[:, :, qt * 128:(qt + 1) * 128], in_=A[:],
                transpose=True)
            state.pop(i)
            if qt == NQ // 128 - 1:
                # rden: [128q,4qt] -> transpose -> [1, 512] -> bcast [128, 512]
                rd4 = sb_st.tile([128, 4], F32, name="rd4", tag="rd4")
                nc.vector.reciprocal(rd4[:], den4[:])
                rdT_ps = ps_s1.tile([1, 4, 128], F32, name="rdT", tag="s1")
                for q4 in range(4):
                    nc.tensor.transpose(rdT_ps[:, q4], rd4[:, q4:q4 + 1],
                                        ident_f32[:])
                rdT = sb_rd.tile([1, NQ], F32, name="rdT_sb", tag="rdT")
                nc.vector.tensor_copy(rdT[:], rdT_ps.rearrange("p a b -> p (a b)"))
                rd_b = sb_rd.tile([128, NQ], F32, name="rd_b", tag="rd_b")
                nc.gpsimd.partition_broadcast(rd_b[:], rdT[:])
                av = ps_av.tile([128, NQ], F32, name="av")
                for kb in range(T // 128):
                    nc.tensor.matmul(av[r0:r0 + DH, :],
                                     lhsT=V_sb[:, kb, h * DH:(h + 1) * DH],
                                     rhs=AT[:, kb],
                                     start=(kb == 0), stop=(kb == T // 128 - 1))
                tail2_state[i] = (av, rd_b, mb, r0)

        def emit_tail2(i):
            if i not in tail2_state:
                return
            av, rd_b, mb, r0 = tail2_state.pop(i)
            nc.vector.tensor_mul(ATTN_mbs[mb][r0:r0 + 64, :],
                                 av[r0:r0 + DH, :], rd_b[r0:r0 + DH, :])

        n = len(units)
        for i in range(n + 2):
            if i < n:
                front_half(i, 0)
            if 0 <= i - 1 < n:
                tail1_half(i - 1, 0)
            if i < n:
                front_half(i, 1)
            if 0 <= i - 1 < n:
                tail1_half(i - 1, 1)
            if i > 1:
                emit_tail2(i - 2)


def _phase_c(nc, tc, d, ATTN_mbs, xt0, bias, ones32, ones1, wo_sb, w1_sb):
    """O proj + residual + LN2 + FF + output store."""
    with ExitStack() as cctx:
        sb_ln2 = cctx.enter_context(tc.tile_pool(name="sb_ln2", bufs=2))
        ps_bc2 = cctx.enter_context(tc.tile_pool(name="ps_bc2", bufs=1, space="PSUM"))
        scr2 = cctx.enter_context(tc.tile_pool(name="scr2", bufs=2))
        sb_u = cctx.enter_context(tc.tile_pool(name="sb_u", bufs=1))
        wstr2 = cctx.enter_context(tc.tile_pool(name="wstr2", bufs=4))
        ps_stat2 = cctx.enter_context(tc.tile_pool(name="ps_stat2", bufs=1, space="PSUM"))
        ps_mm2 = cctx.enter_context(tc.tile_pool(name="ps_mm2", bufs=4, space="PSUM"))

        u_sb = sb_u.tile([128, KD, NQ], F32R, name="u_sb")
        wo_v = wo_sb.rearrange("p (k mb mm) -> p k mb mm", k=KD, mm=128)
        for m in range(KD):
            op = ps_mm2.tile([128, NQ], F32, name="op", tag="mm")
            for k in range(KD):
                nc.tensor.matmul(op[:], lhsT=wo_v[:, k, m], rhs=ATTN_mbs[k][:],
                                 start=(k == 0), stop=(k == KD - 1))
            upre = scr2.tile([128, NQ], F32, name="upre", tag="scr")
            nc.vector.tensor_add(upre[:], op[:], xt0[:, m])
            nc.scalar.activation(u_sb[:, m], upre[:], AF.Identity,
                                 bias=bias["bo"][:, m:m + 1])
        uh = _ln_chunk(nc, sb_ln2, ps_bc2, scr2, ps_stat2, sb_u, u_sb, ones32,
                       ones32, ones1, NQ, out_dt=F16)
        H_sb = sb_u.tile([128, FF // 128, NQ], F16, name="H_sb")
        w1_v = w1_sb.rearrange("p (k mb mm) -> p k mb mm", k=KD, mm=128)
        NPRE = (FF // 128) // 2  # m-blocks prefetched in w1_sb
        for m in range(FF // 128):
            if m < NPRE:
                w1 = w1_v[:, :, m]
            else:
                w1t = wstr2.tile([128, KD, 128], F16, name="w1", tag="wsm")
                nc.sync.dma_start(out=w1t[:], in_=_wslice(d, "w1T", m))
                w1 = w1t
            h1 = ps_mm2.tile([128, NQ], F32, name="h1", tag="mm")
            for k in range(KD):
                nc.tensor.matmul(h1[:], lhsT=w1[:, k], rhs=uh[:, k],
                                 start=(k == 0), stop=(k == KD - 1))
            nc.scalar.activation(H_sb[:, m], h1[:], AF.Gelu,
                                 bias=bias["b1"][:, m:m + 1])
        w2T_v = d["w2T"].rearrange("(kh k p) (mb mm) -> p kh k mb mm",
                                   p=128, k=8, mm=128)
        for m in range(KD):
            f2 = ps_mm2.tile([128, NQ], F32, name="f2", tag="mm")
            for kh in range(4):
                w2 = wstr2.tile([128, 8, 128], F16, name="w2", tag="w2")
                nc.sync.dma_start(out=w2[:], in_=w2T_v[:, kh, :, m])
                for k in range(8):
                    nc.tensor.matmul(f2[:], lhsT=w2[:, k], rhs=H_sb[:, kh * 8 + k],
                                     start=(kh == 0 and k == 0),
                                     stop=(kh == 3 and k == 7))
            opre = scr2.tile([128, NQ], F32, name="opre", tag="scr")
            nc.vector.tensor_add(opre[:], f2[:], u_sb[:, m])
            oout = scr2.tile([128, NQ], F32, name="oout", tag="scr")
            nc.scalar.activation(oout[:], opre[:], AF.Identity,
                                 bias=bias["b2"][:, m:m + 1])
            nc.sync.dma_start(out=d["yT"][m * 128:(m + 1) * 128, :], in_=oout[:])


def _body(nc, tc, d):
    ctx = ExitStack()
    with ctx:
        const = ctx.enter_context(tc.tile_pool(name="const", bufs=1))
        ones_blk = const.tile([128, 128], F32, name="ones_blk")
        nc.vector.memset(ones_blk[:], 1.0)
        ones1r = const.tile([1, 128], F32R, name="ones1r")
        nc.vector.tensor_copy(ones1r[:], ones_blk[0:1, :])
        ones1 = ones1r[:]
        ones32 = const.tile([128, 1], F32R, name="ones32")
        nc.vector.tensor_copy(ones32[:], ones_blk[:, 0:1])
        ones16 = const.tile([128, 1], F16, name="ones16")
        nc.vector.tensor_copy(ones16[:], ones_blk[:, 0:1])
        ident = const.tile([128, 128], F16, name="ident")
        make_identity(nc, ident)
        ident_f32 = const.tile([128, 128], F32, name="ident_f32")
        make_identity(nc, ident_f32)

        bias = {}
        for nm, n in [("bq", DIM), ("bk", DIM), ("bo", DIM), ("b1", FF), ("b2", DIM)]:
            t = const.tile([128, n // 128], F32, name=f"sb_{nm}")
            nc.sync.dma_start(out=t[:], in_=d[nm].rearrange("(m p) -> p m", p=128))
            bias[nm] = t

        # long-lived activations
        xt0 = const.tile([128, KD, CHUNK], F16, name="xt0")
        ATTN_mbs = [const.tile([128, NQ], F16, name=f"ATTN_{i}") for i in range(KD)]

        # phase-C weights, prefetched during attention (issued after phase A
        # so they do not delay phase A's own DMA traffic)
        wpre = ctx.enter_context(tc.tile_pool(name="wpre", bufs=1))
        wo_sb = wpre.tile([128, KD * DIM], F16, name="wo_sb")
        w1_sb = wpre.tile([128, KD * (FF // 2)], F16, name="w1_sb")

        with tc.tile_pool(name="attn_mem", bufs=1) as am:
            KT_sb = am.tile([128, KD, T], F16, name="KT_sb")              # 4MB
            V_sb = am.tile([128, T // 128, DIM], F16, name="V_sb")
            QT_sb = am.tile([128, KD, NQ], F16, name="QT_sb")

            _phase_a(nc, tc, d, const, KT_sb, V_sb, QT_sb, xt0, bias, ones16, ones32, ones1)
            nc.sync.dma_start(out=wo_sb.rearrange("p (k n) -> p k n", k=KD),
                              in_=d["woT"].rearrange("(k p) n -> p k n", p=128))
            nc.sync.dma_start(
                out=w1_sb.rearrange("p (k n) -> p k n", k=KD),
                in_=d["w1T"].rearrange("(k p) n -> p k n", p=128)[:, :, 0:FF // 2])
            _phase_b(nc, tc, KT_sb, V_sb, QT_sb, ATTN_mbs, ident, ident_f32)
        _phase_c(nc, tc, d, ATTN_mbs, xt0, bias, ones32, ones1, wo_sb, w1_sb)


def _build():
    nc = bacc.Bacc("TRN2", target_bir_lowering=False, debug=False,
                   num_devices=N_CORES)
    d = {}
    d["xT"] = nc.dram_tensor("xT", [DIM, T], F16, kind="ExternalInput").ap()
    d["wqT"] = nc.dram_tensor("wqT", [DIM, DIM], F16, kind="ExternalInput").ap()
    d["wkT"] = nc.dram_tensor("wkT", [DIM, DIM], F16, kind="ExternalInput").ap()
    d["wvT"] = nc.dram_tensor("wvT", [DIM, DIM], F16, kind="ExternalInput").ap()
    d["woT"] = nc.dram_tensor("woT", [DIM, DIM], F16, kind="ExternalInput").ap()
    d["w1T"] = nc.dram_tensor("w1T", [DIM, FF], F16, kind="ExternalInput").ap()
    d["w2T"] = nc.dram_tensor("w2T", [FF, DIM], F16, kind="ExternalInput").ap()
    for nm, n in [("bq", DIM), ("bk", DIM), ("bo", DIM), ("b1", FF), ("b2", DIM)]:
        d[nm] = nc.dram_tensor(nm, [n], F32, kind="ExternalInput").ap()
    d["yT"] = nc.dram_tensor("yT", [DIM, NQ], F32, kind="ExternalOutput").ap()
    with tile.TileContext(nc) as tc:
        _body(nc, tc, d)
    nc.compile()
    return nc


def _in_maps(inputs):
    x = inputs["x"].astype(np.float32)
    B = x.shape[0]
    w = _prep_weights(inputs)
    per_batch = N_CORES // B
    maps = []
    for c in range(N_CORES):
        b, chunk = divmod(c, per_batch)
        xT = np.ascontiguousarray(np.roll(x[b].T, -chunk * NQ, axis=1)).astype(np.float16)
        m = {"xT": xT}
        m.update(w)
        maps.append(m)
    return maps


def kernel(**inputs) -> np.ndarray:
    inputs = {k: np.asarray(v) for k, v in inputs.items()}
    x = inputs["x"].astype(np.float32)
    B, N, D = x.shape  # (2, 2048, 1024)

    if "nc" not in _cache:
        _cache["nc"] = _build()
    nc = _cache["nc"]

    res = run_bass_kernel_spmd(nc, _in_maps(inputs), core_ids=list(range(N_CORES)))
    per_batch = N_CORES // B
    out = np.empty((B, N, D), dtype=np.float32)
    for c in range(N_CORES):
        b, chunk = divmod(c, per_batch)
        out[b, chunk * NQ:(chunk + 1) * NQ, :] = res.results[c]["yT"].T
    return out
